# revision 18
# baseline (speedup 1.0000x reference)
"""BIDE forward kernel for Trainium2, 8-core data parallel over B.

Math: logit[b, v] = sum_h cos(zlo[b, lo(v), h] + zhi[b, hi(v), h]) where
  zlo = bits(lo) @ W[:, :8].T          (lo = v & 255)
  zhi = bits(hi) @ W[:, 8:].T + r      (hi = v >> 8)
Using cos(p+q) = cos p cos q - sin p sin q, the [256, 256] logits table is
two K=128 matmuls over trig tables of shape [128 h, 256]:
  table = CloT.T @ ChiT - SloT.T @ ShiT   (per batch row)
logZ = log(sum_v exp(table)) (no max subtraction needed: |logit| <= 128 and
realized max ~40, so exp stays in fp32 range), and the output gather
out[b, t] = table[x[b, t]] - logZ is an indirect DMA from a DRAM copy of
the table.

Sin on the scalar engine only accepts [-pi, pi] (verified: it extrapolates
garbage outside), and the DVE has no mod op, so range reduction uses the
round-to-nearest f32->i32 conversion: the z matmul weights are pre-scaled
by 1/2pi so PSUM holds q = z/2pi; then qi = round(q + c'), w = q - qi, and
sin(z + 2pi c') = Sin(w; scale=2pi, bias=2pi c') with |2pi w + bias| <= pi.

Each core handles 2 of the 16 batch rows; zero cross-core communication.
"""

import numpy as np
import ml_dtypes
from contextlib import ExitStack

import concourse.bacc as bacc
import concourse.bass as bass
from concourse import mybir
from concourse.bass_utils import run_bass_kernel_spmd
from concourse.tile import TileContext

F32 = mybir.dt.float32
BF16 = mybir.dt.bfloat16
I32 = mybir.dt.int32

PI = float(np.float32(np.pi))
HALF_PI = float(np.float32(np.pi / 2.0))
TWO_PI = float(np.float32(2.0 * np.pi))
INV_2PI = 1.0 / (2.0 * np.pi)
# logits for these inputs peak at ~89 (exp overflows fp32) and the ACT Ln
# spline is only valid to 2^64; shift exp by a constant and add it back
EXP_SHIFT = 60.0

N_CORES = 8
B, H, T = 16, 128, 4096
BPC = B // N_CORES  # batch rows per core (2)


def _build():
    nc = bacc.Bacc("TRN2", target_bir_lowering=False, debug=False)

    # lhsT for the z matmuls, one 128-col group per (b, half):
    # rows 0-7 W_hi bits, 8-15 W_lo residual, 16 r_hi, 17 r_lo (hi half only)
    wp = nc.dram_tensor("wp", [18, 512], BF16, kind="ExternalInput")
    # bit-plane enumeration of v in [0, 256): rows 0-7 and 8-15 = (v>>k)&1,
    # rows 16-17 = 1.0 (carries r into zhi)
    bits = nc.dram_tensor("bits", [18, 256], BF16, kind="ExternalInput")
    # x indices: col 0:32 = x[b0] as [128, 32] (t = 32p + j), col 32:64 = x[b1]
    xv = nc.dram_tensor("xv", [128, 64], I32, kind="ExternalInput")
    # negsel[k, 128b + m] = -1 if k == b else 0  (broadcast of -logZ_b)
    negsel_in = nc.dram_tensor("negsel", [2, 256], F32, kind="ExternalInput")
    out = nc.dram_tensor("out", [BPC, T], F32, kind="ExternalOutput")

    with ExitStack() as ctx:
        tc = ctx.enter_context(TileContext(nc))
        sb = ctx.enter_context(tc.tile_pool(name="sb", bufs=1))
        ps_z = ctx.enter_context(tc.tile_pool(name="ps_z", bufs=2, space="PSUM"))
        ps_t = ctx.enter_context(tc.tile_pool(name="ps_t", bufs=2, space="PSUM"))
        ps_s = ctx.enter_context(tc.tile_pool(name="ps_s", bufs=1, space="PSUM"))
        dram = ctx.enter_context(tc.tile_pool(name="dram", bufs=1, space="DRAM"))

        # ---- input loads
        wp_sb = sb.tile([18, 512], BF16, tag="wp")
        bits_sb = sb.tile([18, 256], BF16, tag="bits")
        xv_sb = sb.tile([128, 64], I32, tag="xv")
        nc.sync.dma_start(out=wp_sb[:], in_=wp[:])
        nc.sync.dma_start(out=bits_sb[:], in_=bits[:])
        nc.sync.dma_start(out=xv_sb[:], in_=xv[:])

        # ---- constants
        ones = sb.tile([128, 1], F32, tag="ones")
        nc.vector.memset(ones[:], 1.0)
        # per-partition bias tile for Sin (const-AP registry only has 0/1)
        pio2 = sb.tile([128, 1], F32, tag="pio2")
        nc.vector.memset(pio2[:], HALF_PI)
        neg_shift = sb.tile([128, 1], F32, tag="neg_shift")
        nc.vector.memset(neg_shift[:], -EXP_SHIFT)
        negsel = sb.tile([2, 256], F32, tag="negsel")
        nc.sync.dma_start(out=negsel[:], in_=negsel_in[:])

        # ---- q matmuls: q = z/2pi (weights pre-scaled by 1/2pi), [b0|b1]
        qlo_ps = ps_z.tile([128, 512], F32, tag="z")
        qhi_ps = ps_z.tile([128, 512], F32, tag="z")
        for b in range(BPC):
            nc.tensor.matmul(
                out=qlo_ps[:, 256 * b : 256 * b + 256],
                lhsT=wp_sb[:, 128 * (2 * b) : 128 * (2 * b) + 128],
                rhs=bits_sb[:],
                start=True,
                stop=True,
            )
            nc.tensor.matmul(
                out=qhi_ps[:, 256 * b : 256 * b + 256],
                lhsT=wp_sb[:, 128 * (2 * b + 1) : 128 * (2 * b + 1) + 128],
                rhs=bits_sb[:],
                start=True,
                stop=True,
            )

        # ---- range reduction: qi = round(q + c') (f32->i32 rounds to
        # nearest), w = q - qi, so 2pi*w + 2pi*c' = z + 2pi*c' mod 2pi
        w_a = sb.tile([128, 512], F32, tag="w_a")  # zlo, c'=1/4 -> cos
        w_b = sb.tile([128, 512], F32, tag="w_b")  # zhi, c'=1/4 -> cos
        w_c = sb.tile([128, 512], F32, tag="w_c")  # zlo, c'=0  -> sin
        w_d = sb.tile([128, 512], F32, tag="w_d")  # zhi, c'=0  -> sin
        for i, (w_t, q_ps, cp) in enumerate((
            (w_a, qlo_ps, 0.25),
            (w_b, qhi_ps, 0.25),
            (w_c, qlo_ps, 0.0),
            (w_d, qhi_ps, 0.0),
        )):
            qi_t = sb.tile([128, 512], I32, tag=f"qi{i}")
            if cp == 0.0:
                nc.vector.tensor_copy(out=qi_t[:], in_=q_ps[:])
            else:
                nc.vector.tensor_scalar(
                    out=qi_t[:], in0=q_ps[:], scalar1=cp, scalar2=None,
                    op0=mybir.AluOpType.add,
                )
            nc.vector.tensor_tensor(
                out=w_t[:], in0=q_ps[:], in1=qi_t[:], op=mybir.AluOpType.subtract,
            )

        # ---- trig (ACT): Sin(scale*w + bias), |arg| <= pi
        t_a = sb.tile([128, 512], F32, tag="t_a")  # cos(zlo)
        t_b = sb.tile([128, 512], F32, tag="t_b")  # cos(zhi)
        t_c = sb.tile([128, 512], F32, tag="t_c")  # sin(zlo)
        t_d = sb.tile([128, 512], F32, tag="t_d")  # -sin(zhi) (scale=-2pi)
        for t_t, w_t, scale, bias in (
            (t_a, w_a, TWO_PI, pio2),
            (t_b, w_b, TWO_PI, pio2),
            (t_c, w_c, TWO_PI, 0.0),
            (t_d, w_d, -TWO_PI, 0.0),
        ):
            nc.scalar.activation(
                out=t_t[:], in_=w_t[:],
                func=mybir.ActivationFunctionType.Sin,
                bias=bias if isinstance(bias, float) else bias[:],
                scale=scale,
            )

        # ---- per-b pipeline: table matmuls -> copy/DMA + exp/sum -> gather
        tb_ps = []
        tb_sb = []
        e_sb = []
        g_sb = []
        tbl_dram = []
        sums2 = sb.tile([128, 2], F32, tag="sums2")
        for b in range(BPC):
            bs = slice(256 * b, 256 * b + 256)
            t_ps = ps_t.tile([128, 512], F32, tag="tb")
            tb_ps.append(t_ps)
            # table[hi, lo] = sum_h cos(zhi)cos(zlo) - sin(zhi)sin(zlo)
            #   = A.T@B pairing: lhsT 128-col chunk of hi, rhs full 256 lo
            for c in range(2):
                cs = slice(256 * c, 256 * c + 256)
                hi_s = slice(256 * b + 128 * c, 256 * b + 128 * c + 128)
                nc.tensor.matmul(
                    out=t_ps[:, cs], lhsT=t_b[:, hi_s], rhs=t_a[:, bs],
                    start=True, stop=False,
                )
                nc.tensor.matmul(
                    out=t_ps[:, cs], lhsT=t_d[:, hi_s], rhs=t_c[:, bs],
                    start=False, stop=True,
                )
            # raw table to SBUF (DMA cannot read PSUM), then to DRAM
            t_sb = sb.tile([128, 512], F32, tag=f"tsb{b}")
            tb_sb.append(t_sb)
            nc.vector.tensor_copy(out=t_sb[:], in_=t_ps[:])
            tbl = dram.tile([65536, 1], F32, tag=f"tbl{b}")
            tbl_dram.append(tbl)
            for c in range(2):
                dst = tbl[32768 * c : 32768 * (c + 1), 0:1].rearrange(
                    "(p n) one -> p (n one)", p=128
                )
                nc.sync.dma_start(out=dst, in_=t_sb[:, 256 * c : 256 * c + 256])
            # gather: g[p, j] = table[x[b, 32p + j]]; the indirect DMA
            # consumes one offset per partition, so one call per column j
            g_t = sb.tile([128, 32], F32, tag=f"g{b}")
            g_sb.append(g_t)
            for j in range(32):
                nc.gpsimd.indirect_dma_start(
                    out=g_t[:, j : j + 1],
                    out_offset=None,
                    in_=tbl[:],
                    in_offset=bass.IndirectOffsetOnAxis(
                        ap=xv_sb[:, 32 * b + j : 32 * b + j + 1], axis=0
                    ),
                )
            # exp + row sums for the partition function; EXP_SHIFT keeps
            # exp and the ln input inside fp32 / ACT-spline range
            e_t = sb.tile([128, 512], F32, tag=f"e{b}")
            e_sb.append(e_t)
            nc.scalar.activation(
                out=e_t[:], in_=t_ps[:], func=mybir.ActivationFunctionType.Exp,
                bias=neg_shift[:],
            )
            nc.vector.reduce_sum(
                sums2[:, b : b + 1], e_t[:], axis=mybir.AxisListType.X
            )

        # ---- logZ_b = ln(sum_v exp): partition sum via ones-matmul
        s_ps = ps_s.tile([2, 1], F32, tag="sps")
        nc.tensor.matmul(out=s_ps[:], lhsT=sums2[:], rhs=ones[:], start=True, stop=True)
        logz2 = sb.tile([2, 1], F32, tag="logz2")
        nc.scalar.activation(
            out=logz2[:], in_=s_ps[:], func=mybir.ActivationFunctionType.Ln,
        )

        # ---- out[b, t] = gathered - logZ_b, broadcast via negsel matmul
        for b in range(BPC):
            nz_ps = ps_s.tile([128, 1], F32, tag=f"nz{b}")
            nc.tensor.matmul(
                out=nz_ps[:],
                lhsT=negsel[:, 128 * b : 128 * b + 128],
                rhs=logz2[:],
                start=True, stop=True,
            )
            nz_sb = sb.tile([128, 1], F32, tag=f"nz{b}")
            nc.vector.tensor_copy(out=nz_sb[:], in_=nz_ps[:])
            o_t = sb.tile([128, 32], F32, tag=f"o{b}")
            nc.vector.tensor_scalar(
                out=o_t[:], in0=g_sb[b][:], scalar1=nz_sb[:], scalar2=-EXP_SHIFT,
                op0=mybir.AluOpType.add, op1=mybir.AluOpType.add,
            )
            dst = out[b, :].rearrange("(p j) -> p j", p=128)
            nc.sync.dma_start(out=dst, in_=o_t[:])

    nc.finalize()
    return nc


_NC = None


def _get_nc():
    global _NC
    if _NC is None:
        _NC = _build()
    return _NC


def _bf16_split(a):
    """Return (hi, lo) bf16 arrays with hi + lo ~= a (fp32)."""
    hi = a.astype(ml_dtypes.bfloat16)
    lo = (a - hi.astype(np.float32)).astype(ml_dtypes.bfloat16)
    return hi, lo


def _make_in_maps(x, W, r):
    x = np.asarray(x, dtype=np.int32)
    W = np.asarray(W, dtype=np.float32)
    r = np.asarray(r, dtype=np.float32)

    v = np.arange(256, dtype=np.int32)
    k = np.arange(8, dtype=np.int32)
    bitplanes = ((v[None, :] >> k[:, None]) & 1).astype(np.float32)  # [8, 256]
    bits = np.ones((18, 256), dtype=np.float32)
    bits[0:8] = bitplanes
    bits[8:16] = bitplanes
    bits = bits.astype(ml_dtypes.bfloat16)

    negsel = np.zeros((2, 256), dtype=np.float32)
    negsel[0, 0:128] = -1.0
    negsel[1, 128:256] = -1.0

    in_maps = []
    for core in range(N_CORES):
        wp = np.zeros((18, 512), dtype=ml_dtypes.bfloat16)
        xvs = []
        for b_loc in range(BPC):
            b = BPC * core + b_loc
            for half in range(2):
                g = 2 * b_loc + half
                cs = slice(128 * g, 128 * g + 128)
                w_t = W[b, :, 8 * half : 8 * half + 8].T * INV_2PI  # [8, 128]
                w_hi, w_lo = _bf16_split(w_t.astype(np.float32))
                wp[0:8, cs] = w_hi
                wp[8:16, cs] = w_lo
                if half == 1:
                    r_hi, r_lo = _bf16_split((r[b] * INV_2PI).astype(np.float32))
                    wp[16, cs] = r_hi
                    wp[17, cs] = r_lo
            xvs.append(x[b].reshape(128, 32))
        in_maps.append(
            {
                "wp": wp,
                "bits": bits,
                "xv": np.concatenate(xvs, axis=1).astype(np.int32),
                "negsel": negsel,
            }
        )
    return in_maps


def _run(x, W, r, trace=False):
    nc = _get_nc()
    in_maps = _make_in_maps(x, W, r)
    res = run_bass_kernel_spmd(nc, in_maps, core_ids=list(range(N_CORES)), trace=trace)
    out = np.concatenate([res.results[c]["out"] for c in range(N_CORES)], axis=0)
    return out.astype(np.float32), res


def kernel(x, W, r):
    out, _ = _run(x, W, r)
    return out


def kernel_traced(x, W, r):
    out, res = _run(x, W, r, trace=True)
    return out, res


# revision 22
# speedup vs baseline: 1.6239x; 1.6239x over previous
"""BIDE forward kernel for Trainium2, 8-core data parallel over B.

Math: logit[b, v] = sum_h cos(zlo[b, lo(v), h] + zhi[b, hi(v), h]) where
  zlo = bits(lo) @ W[:, :8].T          (lo = v & 255)
  zhi = bits(hi) @ W[:, 8:].T + r      (hi = v >> 8)
Using cos(p+q) = cos p cos q - sin p sin q, the [256, 256] logits table is
two K=128 matmuls over trig tables of shape [128 h, 256]:
  table = CloT.T @ ChiT - SloT.T @ ShiT   (per batch row)
logZ = log(sum_v exp(table)) (no max subtraction needed: |logit| <= 128 and
realized max ~40, so exp stays in fp32 range), and the output gather
out[b, t] = table[x[b, t]] - logZ is an indirect DMA from a DRAM copy of
the table.

Sin on the scalar engine only accepts [-pi, pi] (verified: it extrapolates
garbage outside), and the DVE has no mod op, so range reduction uses the
round-to-nearest f32->i32 conversion: the z matmul weights are pre-scaled
by 1/2pi so PSUM holds q = z/2pi; then qi = round(q + c'), w = q - qi, and
sin(z + 2pi c') = Sin(w; scale=2pi, bias=2pi c') with |2pi w + bias| <= pi.

Each core handles 2 of the 16 batch rows; zero cross-core communication.
"""

import numpy as np
import ml_dtypes
from contextlib import ExitStack

import concourse.bacc as bacc
import concourse.bass as bass
from concourse import mybir
from concourse.bass_utils import run_bass_kernel_spmd
from concourse.tile import TileContext

F32 = mybir.dt.float32
BF16 = mybir.dt.bfloat16
I32 = mybir.dt.int32

PI = float(np.float32(np.pi))
HALF_PI = float(np.float32(np.pi / 2.0))
TWO_PI = float(np.float32(2.0 * np.pi))
INV_2PI = 1.0 / (2.0 * np.pi)
# logits for these inputs peak at ~89 (exp overflows fp32) and the ACT Ln
# spline is only valid to 2^64; shift exp by a constant and add it back
EXP_SHIFT = 60.0

N_CORES = 8
B, H, T = 16, 128, 4096
BPC = B // N_CORES  # batch rows per core (2)


def _build():
    nc = bacc.Bacc("TRN2", target_bir_lowering=False, debug=False)

    # lhsT for the z matmuls, one 128-col group per (b, half):
    # rows 0-7 W_hi bits, 8-15 W_lo residual, 16 r_hi, 17 r_lo (hi half only)
    wp = nc.dram_tensor("wp", [18, 512], BF16, kind="ExternalInput")
    # bit-plane enumeration of v in [0, 256): rows 0-7 and 8-15 = (v>>k)&1,
    # rows 16-17 = 1.0 (carries r into zhi)
    bits = nc.dram_tensor("bits", [18, 256], BF16, kind="ExternalInput")
    # x indices for the gather: block b at cols [32b, 32b+32), laid out so
    # the indirect DMA's partition-major offset walk (i = s*128 + p) visits
    # t in order: xv[p, 32b + s] = x[b, 128s + p]
    xv = nc.dram_tensor("xv", [128, 64], I32, kind="ExternalInput")
    # negsel[k, 128b + m] = -1 if k == b else 0  (broadcast of -logZ_b)
    negsel_in = nc.dram_tensor("negsel", [2, 256], F32, kind="ExternalInput")
    out = nc.dram_tensor("out", [BPC, T], F32, kind="ExternalOutput")

    with ExitStack() as ctx:
        tc = ctx.enter_context(TileContext(nc))
        sb = ctx.enter_context(tc.tile_pool(name="sb", bufs=1))
        ps_z = ctx.enter_context(tc.tile_pool(name="ps_z", bufs=2, space="PSUM"))
        ps_t = ctx.enter_context(tc.tile_pool(name="ps_t", bufs=2, space="PSUM"))
        ps_s = ctx.enter_context(tc.tile_pool(name="ps_s", bufs=1, space="PSUM"))
        dram = ctx.enter_context(tc.tile_pool(name="dram", bufs=1, space="DRAM"))

        # ---- input loads
        wp_sb = sb.tile([18, 512], BF16, tag="wp")
        bits_sb = sb.tile([18, 256], BF16, tag="bits")
        xv_sb = sb.tile([128, 64], I32, tag="xv")
        nc.sync.dma_start(out=wp_sb[:], in_=wp[:])
        nc.sync.dma_start(out=bits_sb[:], in_=bits[:])
        nc.sync.dma_start(out=xv_sb[:], in_=xv[:])

        # ---- constants
        ones = sb.tile([128, 1], F32, tag="ones")
        nc.vector.memset(ones[:], 1.0)
        # per-partition bias tile for Sin (const-AP registry only has 0/1)
        pio2 = sb.tile([128, 1], F32, tag="pio2")
        nc.vector.memset(pio2[:], HALF_PI)
        neg_shift = sb.tile([128, 1], F32, tag="neg_shift")
        nc.vector.memset(neg_shift[:], -EXP_SHIFT)
        negsel = sb.tile([2, 256], F32, tag="negsel")
        nc.sync.dma_start(out=negsel[:], in_=negsel_in[:])

        # ---- q matmuls: q = z/2pi (weights pre-scaled by 1/2pi), [b0|b1]
        qlo_ps = ps_z.tile([128, 512], F32, tag="z")
        qhi_ps = ps_z.tile([128, 512], F32, tag="z")
        for b in range(BPC):
            nc.tensor.matmul(
                out=qlo_ps[:, 256 * b : 256 * b + 256],
                lhsT=wp_sb[:, 128 * (2 * b) : 128 * (2 * b) + 128],
                rhs=bits_sb[:],
                start=True,
                stop=True,
            )
            nc.tensor.matmul(
                out=qhi_ps[:, 256 * b : 256 * b + 256],
                lhsT=wp_sb[:, 128 * (2 * b + 1) : 128 * (2 * b + 1) + 128],
                rhs=bits_sb[:],
                start=True,
                stop=True,
            )

        # ---- range reduction: qi = round(q + c') (f32->i32 rounds to
        # nearest), w = q - qi, so 2pi*w + 2pi*c' = z + 2pi*c' mod 2pi
        w_a = sb.tile([128, 512], F32, tag="w_a")  # zlo, c'=1/4 -> cos
        w_b = sb.tile([128, 512], F32, tag="w_b")  # zhi, c'=1/4 -> cos
        w_c = sb.tile([128, 512], F32, tag="w_c")  # zlo, c'=0  -> sin
        w_d = sb.tile([128, 512], F32, tag="w_d")  # zhi, c'=0  -> sin
        for i, (w_t, q_ps, cp) in enumerate((
            (w_a, qlo_ps, 0.25),
            (w_b, qhi_ps, 0.25),
            (w_c, qlo_ps, 0.0),
            (w_d, qhi_ps, 0.0),
        )):
            qi_t = sb.tile([128, 512], I32, tag=f"qi{i}")
            if cp == 0.0:
                nc.vector.tensor_copy(out=qi_t[:], in_=q_ps[:])
            else:
                nc.vector.tensor_scalar(
                    out=qi_t[:], in0=q_ps[:], scalar1=cp, scalar2=None,
                    op0=mybir.AluOpType.add,
                )
            nc.vector.tensor_tensor(
                out=w_t[:], in0=q_ps[:], in1=qi_t[:], op=mybir.AluOpType.subtract,
            )

        # ---- trig (ACT): Sin(scale*w + bias), |arg| <= pi
        t_a = sb.tile([128, 512], F32, tag="t_a")  # cos(zlo)
        t_b = sb.tile([128, 512], F32, tag="t_b")  # cos(zhi)
        t_c = sb.tile([128, 512], F32, tag="t_c")  # sin(zlo)
        t_d = sb.tile([128, 512], F32, tag="t_d")  # -sin(zhi) (scale=-2pi)
        for t_t, w_t, scale, bias in (
            (t_a, w_a, TWO_PI, pio2),
            (t_b, w_b, TWO_PI, pio2),
            (t_c, w_c, TWO_PI, 0.0),
            (t_d, w_d, -TWO_PI, 0.0),
        ):
            nc.scalar.activation(
                out=t_t[:], in_=w_t[:],
                func=mybir.ActivationFunctionType.Sin,
                bias=bias if isinstance(bias, float) else bias[:],
                scale=scale,
            )

        # ---- per-b pipeline: table matmuls -> copy/DMA + exp/sum -> gather
        tb_ps = []
        tb_sb = []
        e_sb = []
        g_sb = []
        tbl_dram = []
        sums2 = sb.tile([128, 2], F32, tag="sums2")
        for b in range(BPC):
            bs = slice(256 * b, 256 * b + 256)
            t_ps = ps_t.tile([128, 512], F32, tag="tb")
            tb_ps.append(t_ps)
            # table[hi, lo] = sum_h cos(zhi)cos(zlo) - sin(zhi)sin(zlo)
            #   = A.T@B pairing: lhsT 128-col chunk of hi, rhs full 256 lo
            for c in range(2):
                cs = slice(256 * c, 256 * c + 256)
                hi_s = slice(256 * b + 128 * c, 256 * b + 128 * c + 128)
                nc.tensor.matmul(
                    out=t_ps[:, cs], lhsT=t_b[:, hi_s], rhs=t_a[:, bs],
                    start=True, stop=False,
                )
                nc.tensor.matmul(
                    out=t_ps[:, cs], lhsT=t_d[:, hi_s], rhs=t_c[:, bs],
                    start=False, stop=True,
                )
            # raw table to SBUF (DMA cannot read PSUM), then to DRAM
            t_sb = sb.tile([128, 512], F32, tag=f"tsb{b}")
            tb_sb.append(t_sb)
            nc.vector.tensor_copy(out=t_sb[:], in_=t_ps[:])
            tbl = dram.tile([65536, 1], F32, tag=f"tbl{b}")
            tbl_dram.append(tbl)
            for c in range(2):
                dst = tbl[32768 * c : 32768 * (c + 1), 0:1].rearrange(
                    "(p n) one -> p (n one)", p=128
                )
                nc.sync.dma_start(out=dst, in_=t_sb[:, 256 * c : 256 * c + 256])
            # gather all 4096 elements in ONE indirect DMA: a [1, 4096, 1]
            # dest makes the DGE emit one descriptor per element, walking
            # the offset AP partition-major; then redistribute [1, 4096] ->
            # [128, 32] (g2[p, j] = value for t = 32p + j) with a plain DMA
            g_t = sb.tile([1, 4096], F32, tag=f"g{b}")
            nc.gpsimd.indirect_dma_start(
                out=g_t[:].rearrange("p (i one) -> p i one", one=1),
                out_offset=None,
                in_=tbl[:],
                in_offset=bass.IndirectOffsetOnAxis(
                    ap=xv_sb[:, 32 * b : 32 * b + 32], axis=0
                ),
            )
            g2_t = sb.tile([128, 32], F32, tag=f"g2{b}")
            g_sb.append(g2_t)
            nc.sync.dma_start(
                out=g2_t[:],
                in_=g_t[:].rearrange("one (p j) -> one p j", p=128),
            )
            # exp + row sums for the partition function; EXP_SHIFT keeps
            # exp and the ln input inside fp32 / ACT-spline range
            e_t = sb.tile([128, 512], F32, tag=f"e{b}")
            e_sb.append(e_t)
            nc.scalar.activation(
                out=e_t[:], in_=t_ps[:], func=mybir.ActivationFunctionType.Exp,
                bias=neg_shift[:],
            )
            nc.vector.reduce_sum(
                sums2[:, b : b + 1], e_t[:], axis=mybir.AxisListType.X
            )

        # ---- logZ_b = ln(sum_v exp): partition sum via ones-matmul
        s_ps = ps_s.tile([2, 1], F32, tag="sps")
        nc.tensor.matmul(out=s_ps[:], lhsT=sums2[:], rhs=ones[:], start=True, stop=True)
        logz2 = sb.tile([2, 1], F32, tag="logz2")
        nc.scalar.activation(
            out=logz2[:], in_=s_ps[:], func=mybir.ActivationFunctionType.Ln,
        )

        # ---- out[b, t] = gathered - logZ_b, broadcast via negsel matmul
        for b in range(BPC):
            nz_ps = ps_s.tile([128, 1], F32, tag=f"nz{b}")
            nc.tensor.matmul(
                out=nz_ps[:],
                lhsT=negsel[:, 128 * b : 128 * b + 128],
                rhs=logz2[:],
                start=True, stop=True,
            )
            nz_sb = sb.tile([128, 1], F32, tag=f"nz{b}")
            nc.vector.tensor_copy(out=nz_sb[:], in_=nz_ps[:])
            o_t = sb.tile([128, 32], F32, tag=f"o{b}")
            nc.vector.tensor_scalar(
                out=o_t[:], in0=g_sb[b][:], scalar1=nz_sb[:], scalar2=-EXP_SHIFT,
                op0=mybir.AluOpType.add, op1=mybir.AluOpType.add,
            )
            dst = out[b, :].rearrange("(p j) -> p j", p=128)
            nc.sync.dma_start(out=dst, in_=o_t[:])

    nc.finalize()
    return nc


_NC = None


def _get_nc():
    global _NC
    if _NC is None:
        _NC = _build()
    return _NC


def _bf16_split(a):
    """Return (hi, lo) bf16 arrays with hi + lo ~= a (fp32)."""
    hi = a.astype(ml_dtypes.bfloat16)
    lo = (a - hi.astype(np.float32)).astype(ml_dtypes.bfloat16)
    return hi, lo


def _make_in_maps(x, W, r):
    x = np.asarray(x, dtype=np.int32)
    W = np.asarray(W, dtype=np.float32)
    r = np.asarray(r, dtype=np.float32)

    v = np.arange(256, dtype=np.int32)
    k = np.arange(8, dtype=np.int32)
    bitplanes = ((v[None, :] >> k[:, None]) & 1).astype(np.float32)  # [8, 256]
    bits = np.ones((18, 256), dtype=np.float32)
    bits[0:8] = bitplanes
    bits[8:16] = bitplanes
    bits = bits.astype(ml_dtypes.bfloat16)

    negsel = np.zeros((2, 256), dtype=np.float32)
    negsel[0, 0:128] = -1.0
    negsel[1, 128:256] = -1.0

    in_maps = []
    for core in range(N_CORES):
        wp = np.zeros((18, 512), dtype=ml_dtypes.bfloat16)
        xvs = []
        for b_loc in range(BPC):
            b = BPC * core + b_loc
            for half in range(2):
                g = 2 * b_loc + half
                cs = slice(128 * g, 128 * g + 128)
                w_t = W[b, :, 8 * half : 8 * half + 8].T * INV_2PI  # [8, 128]
                w_hi, w_lo = _bf16_split(w_t.astype(np.float32))
                wp[0:8, cs] = w_hi
                wp[8:16, cs] = w_lo
                if half == 1:
                    r_hi, r_lo = _bf16_split((r[b] * INV_2PI).astype(np.float32))
                    wp[16, cs] = r_hi
                    wp[17, cs] = r_lo
            xvs.append(x[b].reshape(32, 128).T)
        in_maps.append(
            {
                "wp": wp,
                "bits": bits,
                "xv": np.concatenate(xvs, axis=1).astype(np.int32),
                "negsel": negsel,
            }
        )
    return in_maps


def _run(x, W, r, trace=False):
    nc = _get_nc()
    in_maps = _make_in_maps(x, W, r)
    res = run_bass_kernel_spmd(nc, in_maps, core_ids=list(range(N_CORES)), trace=trace)
    out = np.concatenate([res.results[c]["out"] for c in range(N_CORES)], axis=0)
    return out.astype(np.float32), res


def kernel(x, W, r):
    out, _ = _run(x, W, r)
    return out


def kernel_traced(x, W, r):
    out, res = _run(x, W, r, trace=True)
    return out, res


# revision 25
# speedup vs baseline: 1.6880x; 1.0395x over previous
"""BIDE forward kernel for Trainium2, 8-core data parallel over B.

Math: logit[b, v] = sum_h cos(zlo[b, lo(v), h] + zhi[b, hi(v), h]) where
  zlo = bits(lo) @ W[:, :8].T          (lo = v & 255)
  zhi = bits(hi) @ W[:, 8:].T + r      (hi = v >> 8)
Using cos(p+q) = cos p cos q - sin p sin q, the [256, 256] logits table is
two K=128 matmuls over trig tables of shape [128 h, 256]:
  table = CloT.T @ ChiT - SloT.T @ ShiT   (per batch row)
logZ = log(sum_v exp(table)) (no max subtraction needed: |logit| <= 128 and
realized max ~40, so exp stays in fp32 range), and the output gather
out[b, t] = table[x[b, t]] - logZ is an indirect DMA from a DRAM copy of
the table.

Sin on the scalar engine only accepts [-pi, pi] (verified: it extrapolates
garbage outside), and the DVE has no mod op, so range reduction uses the
round-to-nearest f32->i32 conversion: the z matmul weights are pre-scaled
by 1/2pi so PSUM holds q = z/2pi; then qi = round(q + c'), w = q - qi, and
sin(z + 2pi c') = Sin(w; scale=2pi, bias=2pi c') with |2pi w + bias| <= pi.

Each core handles 2 of the 16 batch rows; zero cross-core communication.
"""

import numpy as np
import ml_dtypes
from contextlib import ExitStack

import concourse.bacc as bacc
import concourse.bass as bass
from concourse import mybir
from concourse.bass_utils import run_bass_kernel_spmd
from concourse.tile import TileContext

F32 = mybir.dt.float32
BF16 = mybir.dt.bfloat16
I32 = mybir.dt.int32

PI = float(np.float32(np.pi))
HALF_PI = float(np.float32(np.pi / 2.0))
TWO_PI = float(np.float32(2.0 * np.pi))
INV_2PI = 1.0 / (2.0 * np.pi)
# logits for these inputs peak at ~89 (exp overflows fp32) and the ACT Ln
# spline is only valid to 2^64; shift exp by a constant and add it back
EXP_SHIFT = 60.0

N_CORES = 8
B, H, T = 16, 128, 4096
BPC = B // N_CORES  # batch rows per core (2)


def _build():
    nc = bacc.Bacc("TRN2", target_bir_lowering=False, debug=False)

    # lhsT for the z matmuls, one 128-col group per (b, half):
    # rows 0-7 W_hi bits, 8-15 W_lo residual, 16 r_hi, 17 r_lo (hi half only)
    wp = nc.dram_tensor("wp", [18, 512], BF16, kind="ExternalInput")
    # bit-plane enumeration of v in [0, 256): rows 0-7 and 8-15 = (v>>k)&1,
    # rows 16-17 = 1.0 (carries r into zhi)
    bits = nc.dram_tensor("bits", [18, 256], BF16, kind="ExternalInput")
    # x indices for the gather: block b at cols [32b, 32b+32), laid out so
    # the indirect DMA's partition-major offset walk (i = s*128 + p) visits
    # t in order: xv[p, 32b + s] = x[b, 128s + p]
    xv = nc.dram_tensor("xv", [128, 64], I32, kind="ExternalInput")
    # negsel[k, 128b + m] = -1 if k == b else 0  (broadcast of -logZ_b)
    negsel_in = nc.dram_tensor("negsel", [2, 256], F32, kind="ExternalInput")
    out = nc.dram_tensor("out", [BPC, T], F32, kind="ExternalOutput")

    with ExitStack() as ctx:
        tc = ctx.enter_context(TileContext(nc))
        sb = ctx.enter_context(tc.tile_pool(name="sb", bufs=1))
        ps_z = ctx.enter_context(tc.tile_pool(name="ps_z", bufs=2, space="PSUM"))
        ps_t = ctx.enter_context(tc.tile_pool(name="ps_t", bufs=2, space="PSUM"))
        ps_s = ctx.enter_context(tc.tile_pool(name="ps_s", bufs=1, space="PSUM"))
        dram = ctx.enter_context(tc.tile_pool(name="dram", bufs=1, space="DRAM"))

        # ---- input loads
        wp_sb = sb.tile([18, 512], BF16, tag="wp")
        bits_sb = sb.tile([18, 256], BF16, tag="bits")
        xv_sb = sb.tile([128, 64], I32, tag="xv")
        nc.sync.dma_start(out=wp_sb[:], in_=wp[:])
        nc.sync.dma_start(out=bits_sb[:], in_=bits[:])
        nc.sync.dma_start(out=xv_sb[:], in_=xv[:])

        # ---- constants
        ones = sb.tile([128, 1], F32, tag="ones")
        nc.vector.memset(ones[:], 1.0)
        # per-partition bias tile for Sin (const-AP registry only has 0/1)
        pio2 = sb.tile([128, 1], F32, tag="pio2")
        nc.vector.memset(pio2[:], HALF_PI)
        neg_shift = sb.tile([128, 1], F32, tag="neg_shift")
        nc.vector.memset(neg_shift[:], -EXP_SHIFT)
        negsel = sb.tile([2, 256], F32, tag="negsel")
        nc.sync.dma_start(out=negsel[:], in_=negsel_in[:])

        # ---- q matmuls: q = z/2pi (weights pre-scaled by 1/2pi), [b0|b1]
        qlo_ps = ps_z.tile([128, 512], F32, tag="z")
        qhi_ps = ps_z.tile([128, 512], F32, tag="z")
        for b in range(BPC):
            nc.tensor.matmul(
                out=qlo_ps[:, 256 * b : 256 * b + 256],
                lhsT=wp_sb[:, 128 * (2 * b) : 128 * (2 * b) + 128],
                rhs=bits_sb[:],
                start=True,
                stop=True,
            )
            nc.tensor.matmul(
                out=qhi_ps[:, 256 * b : 256 * b + 256],
                lhsT=wp_sb[:, 128 * (2 * b + 1) : 128 * (2 * b + 1) + 128],
                rhs=bits_sb[:],
                start=True,
                stop=True,
            )

        # ---- range reduction: qi = round(q + c') (f32->i32 rounds to
        # nearest), w = q - qi, so 2pi*w + 2pi*c' = z + 2pi*c' mod 2pi
        w_a = sb.tile([128, 512], F32, tag="w_a")  # zlo, c'=1/4 -> cos
        w_b = sb.tile([128, 512], F32, tag="w_b")  # zhi, c'=1/4 -> cos
        w_c = sb.tile([128, 512], F32, tag="w_c")  # zlo, c'=0  -> sin
        w_d = sb.tile([128, 512], F32, tag="w_d")  # zhi, c'=0  -> sin
        for i, (w_t, q_ps, cp) in enumerate((
            (w_a, qlo_ps, 0.25),
            (w_b, qhi_ps, 0.25),
            (w_c, qlo_ps, 0.0),
            (w_d, qhi_ps, 0.0),
        )):
            qi_t = sb.tile([128, 512], I32, tag=f"qi{i}")
            if cp == 0.0:
                nc.vector.tensor_copy(out=qi_t[:], in_=q_ps[:])
            else:
                nc.vector.tensor_scalar(
                    out=qi_t[:], in0=q_ps[:], scalar1=cp, scalar2=None,
                    op0=mybir.AluOpType.add,
                )
            nc.vector.tensor_tensor(
                out=w_t[:], in0=q_ps[:], in1=qi_t[:], op=mybir.AluOpType.subtract,
            )

        # ---- trig (ACT): Sin(scale*w + bias), |arg| <= pi
        t_a = sb.tile([128, 512], F32, tag="t_a")  # cos(zlo)
        t_b = sb.tile([128, 512], F32, tag="t_b")  # cos(zhi)
        t_c = sb.tile([128, 512], F32, tag="t_c")  # sin(zlo)
        t_d = sb.tile([128, 512], F32, tag="t_d")  # -sin(zhi) (scale=-2pi)
        for t_t, w_t, scale, bias in (
            (t_a, w_a, TWO_PI, pio2),
            (t_b, w_b, TWO_PI, pio2),
            (t_c, w_c, TWO_PI, 0.0),
            (t_d, w_d, -TWO_PI, 0.0),
        ):
            nc.scalar.activation(
                out=t_t[:], in_=w_t[:],
                func=mybir.ActivationFunctionType.Sin,
                bias=bias if isinstance(bias, float) else bias[:],
                scale=scale,
            )

        # ---- per-b pipeline: table matmuls -> copy/DMA + exp/sum -> gather
        tb_ps = []
        tb_sb = []
        e_sb = []
        g_sb = []
        tbl_dram = []
        sums2 = sb.tile([128, 2], F32, tag="sums2")
        for b in range(BPC):
            bs = slice(256 * b, 256 * b + 256)
            t_ps = ps_t.tile([128, 512], F32, tag="tb")
            tb_ps.append(t_ps)
            # table[hi, lo] = sum_h cos(zhi)cos(zlo) - sin(zhi)sin(zlo)
            #   = A.T@B pairing: lhsT 128-col chunk of hi, rhs full 256 lo
            for c in range(2):
                cs = slice(256 * c, 256 * c + 256)
                hi_s = slice(256 * b + 128 * c, 256 * b + 128 * c + 128)
                nc.tensor.matmul(
                    out=t_ps[:, cs], lhsT=t_b[:, hi_s], rhs=t_a[:, bs],
                    start=True, stop=False,
                )
                nc.tensor.matmul(
                    out=t_ps[:, cs], lhsT=t_d[:, hi_s], rhs=t_c[:, bs],
                    start=False, stop=True,
                )
            # raw table to SBUF (DMA cannot read PSUM), then to DRAM
            t_sb = sb.tile([128, 512], F32, tag=f"tsb{b}")
            tb_sb.append(t_sb)
            nc.vector.tensor_copy(out=t_sb[:], in_=t_ps[:])
            tbl = dram.tile([65536, 1], F32, tag=f"tbl{b}")
            tbl_dram.append(tbl)
            for c in range(2):
                dst = tbl[32768 * c : 32768 * (c + 1), 0:1].rearrange(
                    "(p n) one -> p (n one)", p=128
                )
                nc.sync.dma_start(out=dst, in_=t_sb[:, 256 * c : 256 * c + 256])
            # gather: a [1, N, 1] dest makes the DGE emit one descriptor per
            # element, walking the offset AP partition-major (i = 128s + p).
            # One dest partition = one SBUF port (~6.4ns/element serialized),
            # so split into 4 calls on partitions {0,4,8,12} (+16 for b1),
            # which map to 8 distinct SBUF ports across the two batch rows.
            g_t = sb.tile([32, 1024], F32, tag=f"g{b}")
            for c in range(4):
                row = 4 * c + 16 * b
                nc.gpsimd.indirect_dma_start(
                    out=g_t[row : row + 1, :].rearrange(
                        "one (i x) -> one i x", x=1
                    ),
                    out_offset=None,
                    in_=tbl[:],
                    in_offset=bass.IndirectOffsetOnAxis(
                        ap=xv_sb[:, 32 * b + 8 * c : 32 * b + 8 * c + 8], axis=0
                    ),
                )
            # redistribute to g2[p, j] = value for t = 32p + j: with p =
            # 32c + q, t = 1024c + 32q + j lives at g_t[4c + 16b, 32q + j]
            g2_t = sb.tile([128, 32], F32, tag=f"g2{b}")
            g_sb.append(g2_t)
            for c in range(4):
                nc.sync.dma_start(
                    out=g2_t[32 * c : 32 * c + 32, :],
                    in_=g_t[4 * c + 16 * b : 4 * c + 16 * b + 1, :].rearrange(
                        "one (q j) -> one q j", j=32
                    ),
                )
            # exp + row sums for the partition function; EXP_SHIFT keeps
            # exp and the ln input inside fp32 / ACT-spline range
            e_t = sb.tile([128, 512], F32, tag=f"e{b}")
            e_sb.append(e_t)
            nc.scalar.activation(
                out=e_t[:], in_=t_ps[:], func=mybir.ActivationFunctionType.Exp,
                bias=neg_shift[:],
            )
            nc.vector.reduce_sum(
                sums2[:, b : b + 1], e_t[:], axis=mybir.AxisListType.X
            )

        # ---- logZ_b = ln(sum_v exp): partition sum via ones-matmul
        s_ps = ps_s.tile([2, 1], F32, tag="sps")
        nc.tensor.matmul(out=s_ps[:], lhsT=sums2[:], rhs=ones[:], start=True, stop=True)
        logz2 = sb.tile([2, 1], F32, tag="logz2")
        nc.scalar.activation(
            out=logz2[:], in_=s_ps[:], func=mybir.ActivationFunctionType.Ln,
        )

        # ---- out[b, t] = gathered - logZ_b, broadcast via negsel matmul
        for b in range(BPC):
            nz_ps = ps_s.tile([128, 1], F32, tag=f"nz{b}")
            nc.tensor.matmul(
                out=nz_ps[:],
                lhsT=negsel[:, 128 * b : 128 * b + 128],
                rhs=logz2[:],
                start=True, stop=True,
            )
            nz_sb = sb.tile([128, 1], F32, tag=f"nz{b}")
            nc.vector.tensor_copy(out=nz_sb[:], in_=nz_ps[:])
            o_t = sb.tile([128, 32], F32, tag=f"o{b}")
            nc.vector.tensor_scalar(
                out=o_t[:], in0=g_sb[b][:], scalar1=nz_sb[:], scalar2=-EXP_SHIFT,
                op0=mybir.AluOpType.add, op1=mybir.AluOpType.add,
            )
            dst = out[b, :].rearrange("(p j) -> p j", p=128)
            nc.sync.dma_start(out=dst, in_=o_t[:])

    nc.finalize()
    return nc


_NC = None


def _get_nc():
    global _NC
    if _NC is None:
        _NC = _build()
    return _NC


def _bf16_split(a):
    """Return (hi, lo) bf16 arrays with hi + lo ~= a (fp32)."""
    hi = a.astype(ml_dtypes.bfloat16)
    lo = (a - hi.astype(np.float32)).astype(ml_dtypes.bfloat16)
    return hi, lo


def _make_in_maps(x, W, r):
    x = np.asarray(x, dtype=np.int32)
    W = np.asarray(W, dtype=np.float32)
    r = np.asarray(r, dtype=np.float32)

    v = np.arange(256, dtype=np.int32)
    k = np.arange(8, dtype=np.int32)
    bitplanes = ((v[None, :] >> k[:, None]) & 1).astype(np.float32)  # [8, 256]
    bits = np.ones((18, 256), dtype=np.float32)
    bits[0:8] = bitplanes
    bits[8:16] = bitplanes
    bits = bits.astype(ml_dtypes.bfloat16)

    negsel = np.zeros((2, 256), dtype=np.float32)
    negsel[0, 0:128] = -1.0
    negsel[1, 128:256] = -1.0

    in_maps = []
    for core in range(N_CORES):
        wp = np.zeros((18, 512), dtype=ml_dtypes.bfloat16)
        xvs = []
        for b_loc in range(BPC):
            b = BPC * core + b_loc
            for half in range(2):
                g = 2 * b_loc + half
                cs = slice(128 * g, 128 * g + 128)
                w_t = W[b, :, 8 * half : 8 * half + 8].T * INV_2PI  # [8, 128]
                w_hi, w_lo = _bf16_split(w_t.astype(np.float32))
                wp[0:8, cs] = w_hi
                wp[8:16, cs] = w_lo
                if half == 1:
                    r_hi, r_lo = _bf16_split((r[b] * INV_2PI).astype(np.float32))
                    wp[16, cs] = r_hi
                    wp[17, cs] = r_lo
            xvs.append(x[b].reshape(32, 128).T)
        in_maps.append(
            {
                "wp": wp,
                "bits": bits,
                "xv": np.concatenate(xvs, axis=1).astype(np.int32),
                "negsel": negsel,
            }
        )
    return in_maps


def _run(x, W, r, trace=False):
    nc = _get_nc()
    in_maps = _make_in_maps(x, W, r)
    res = run_bass_kernel_spmd(nc, in_maps, core_ids=list(range(N_CORES)), trace=trace)
    out = np.concatenate([res.results[c]["out"] for c in range(N_CORES)], axis=0)
    return out.astype(np.float32), res


def kernel(x, W, r):
    out, _ = _run(x, W, r)
    return out


def kernel_traced(x, W, r):
    out, res = _run(x, W, r, trace=True)
    return out, res


# revision 27
# speedup vs baseline: 1.8557x; 1.0993x over previous
"""BIDE forward kernel for Trainium2, 8-core data parallel over B.

Math: logit[b, v] = sum_h cos(zlo[b, lo(v), h] + zhi[b, hi(v), h]) where
  zlo = bits(lo) @ W[:, :8].T          (lo = v & 255)
  zhi = bits(hi) @ W[:, 8:].T + r      (hi = v >> 8)
Using cos(p+q) = cos p cos q - sin p sin q, the [256, 256] logits table is
two K=128 matmuls over trig tables of shape [128 h, 256]:
  table = CloT.T @ ChiT - SloT.T @ ShiT   (per batch row)
logZ = log(sum_v exp(table)) (no max subtraction needed: |logit| <= 128 and
realized max ~40, so exp stays in fp32 range), and the output gather
out[b, t] = table[x[b, t]] - logZ is an indirect DMA from a DRAM copy of
the table.

Sin on the scalar engine only accepts [-pi, pi] (verified: it extrapolates
garbage outside), and the DVE has no mod op, so range reduction uses the
round-to-nearest f32->i32 conversion: the z matmul weights are pre-scaled
by 1/2pi so PSUM holds q = z/2pi; then qi = round(q + c'), w = q - qi, and
sin(z + 2pi c') = Sin(w; scale=2pi, bias=2pi c') with |2pi w + bias| <= pi.

Each core handles 2 of the 16 batch rows; zero cross-core communication.
"""

import numpy as np
import ml_dtypes
from contextlib import ExitStack

import concourse.bacc as bacc
import concourse.bass as bass
from concourse import mybir
from concourse.bass_utils import run_bass_kernel_spmd
from concourse.tile import TileContext

F32 = mybir.dt.float32
BF16 = mybir.dt.bfloat16
I32 = mybir.dt.int32

PI = float(np.float32(np.pi))
HALF_PI = float(np.float32(np.pi / 2.0))
TWO_PI = float(np.float32(2.0 * np.pi))
INV_2PI = 1.0 / (2.0 * np.pi)
# logits for these inputs peak at ~89 (exp overflows fp32) and the ACT Ln
# spline is only valid to 2^64; shift exp by a constant and add it back
EXP_SHIFT = 60.0

N_CORES = 8
B, H, T = 16, 128, 4096
BPC = B // N_CORES  # batch rows per core (2)


def _build():
    nc = bacc.Bacc("TRN2", target_bir_lowering=False, debug=False)

    # lhsT for the z matmuls, one 128-col group per (b, half):
    # rows 0-7 W_hi bits, 8-15 W_lo residual, 16 r_hi, 17 r_lo (hi half only)
    wp = nc.dram_tensor("wp", [18, 512], BF16, kind="ExternalInput")
    # bit-plane enumeration of v in [0, 256): rows 0-7 and 8-15 = (v>>k)&1,
    # rows 16-17 = 1.0 (carries r into zhi)
    bits = nc.dram_tensor("bits", [18, 256], BF16, kind="ExternalInput")
    # x indices for the gather: block b at cols [32b, 32b+32), laid out so
    # the indirect DMA's partition-major offset walk (i = s*128 + p) visits
    # t in order: xv[p, 32b + s] = x[b, 128s + p]
    xv = nc.dram_tensor("xv", [128, 64], I32, kind="ExternalInput")
    # negsel[k, 128b + m] = -1 if k == b else 0  (broadcast of -logZ_b)
    negsel_in = nc.dram_tensor("negsel", [2, 256], F32, kind="ExternalInput")
    out = nc.dram_tensor("out", [BPC, T], F32, kind="ExternalOutput")

    with ExitStack() as ctx:
        tc = ctx.enter_context(TileContext(nc))
        sb = ctx.enter_context(tc.tile_pool(name="sb", bufs=1))
        ps_z = ctx.enter_context(tc.tile_pool(name="ps_z", bufs=2, space="PSUM"))
        ps_t = ctx.enter_context(tc.tile_pool(name="ps_t", bufs=2, space="PSUM"))
        ps_s = ctx.enter_context(tc.tile_pool(name="ps_s", bufs=1, space="PSUM"))
        dram = ctx.enter_context(tc.tile_pool(name="dram", bufs=1, space="DRAM"))

        # ---- input loads
        wp_sb = sb.tile([18, 512], BF16, tag="wp")
        bits_sb = sb.tile([18, 256], BF16, tag="bits")
        xv_sb = sb.tile([128, 64], I32, tag="xv")
        nc.sync.dma_start(out=wp_sb[:], in_=wp[:])
        nc.sync.dma_start(out=bits_sb[:], in_=bits[:])
        nc.sync.dma_start(out=xv_sb[:], in_=xv[:])

        # ---- constants
        ones = sb.tile([128, 1], F32, tag="ones")
        nc.vector.memset(ones[:], 1.0)
        # per-partition bias tile for Sin (const-AP registry only has 0/1)
        pio2 = sb.tile([128, 1], F32, tag="pio2")
        nc.vector.memset(pio2[:], HALF_PI)
        neg_shift = sb.tile([128, 1], F32, tag="neg_shift")
        nc.vector.memset(neg_shift[:], -EXP_SHIFT)
        negsel = sb.tile([2, 256], F32, tag="negsel")
        nc.sync.dma_start(out=negsel[:], in_=negsel_in[:])

        # ---- q matmuls: q = z/2pi (weights pre-scaled by 1/2pi), [b0|b1]
        qlo_ps = ps_z.tile([128, 512], F32, tag="z")
        qhi_ps = ps_z.tile([128, 512], F32, tag="z")
        for b in range(BPC):
            nc.tensor.matmul(
                out=qlo_ps[:, 256 * b : 256 * b + 256],
                lhsT=wp_sb[:, 128 * (2 * b) : 128 * (2 * b) + 128],
                rhs=bits_sb[:],
                start=True,
                stop=True,
            )
            nc.tensor.matmul(
                out=qhi_ps[:, 256 * b : 256 * b + 256],
                lhsT=wp_sb[:, 128 * (2 * b + 1) : 128 * (2 * b + 1) + 128],
                rhs=bits_sb[:],
                start=True,
                stop=True,
            )

        # ---- range reduction: qi = round(q + c') (f32->i32 rounds to
        # nearest), w = q - qi, so 2pi*w + 2pi*c' = z + 2pi*c' mod 2pi
        w_a = sb.tile([128, 512], F32, tag="w_a")  # zlo, c'=1/4 -> cos
        w_b = sb.tile([128, 512], F32, tag="w_b")  # zhi, c'=1/4 -> cos
        w_c = sb.tile([128, 512], F32, tag="w_c")  # zlo, c'=0  -> sin
        w_d = sb.tile([128, 512], F32, tag="w_d")  # zhi, c'=0  -> sin
        for i, (w_t, q_ps, cp) in enumerate((
            (w_a, qlo_ps, 0.25),
            (w_b, qhi_ps, 0.25),
            (w_c, qlo_ps, 0.0),
            (w_d, qhi_ps, 0.0),
        )):
            qi_t = sb.tile([128, 512], I32, tag=f"qi{i}")
            if cp == 0.0:
                nc.vector.tensor_copy(out=qi_t[:], in_=q_ps[:])
            else:
                nc.vector.tensor_scalar(
                    out=qi_t[:], in0=q_ps[:], scalar1=cp, scalar2=None,
                    op0=mybir.AluOpType.add,
                )
            nc.vector.tensor_tensor(
                out=w_t[:], in0=q_ps[:], in1=qi_t[:], op=mybir.AluOpType.subtract,
            )

        # ---- trig (ACT): Sin(scale*w + bias), |arg| <= pi
        F32R = mybir.dt.float32r
        t_a = sb.tile([128, 512], F32R, tag="t_a")  # cos(zlo)
        t_b = sb.tile([128, 512], F32R, tag="t_b")  # cos(zhi)
        t_c = sb.tile([128, 512], F32R, tag="t_c")  # sin(zlo)
        t_d = sb.tile([128, 512], F32R, tag="t_d")  # -sin(zhi) (scale=-2pi)
        for t_t, w_t, scale, bias in (
            (t_a, w_a, TWO_PI, pio2),
            (t_b, w_b, TWO_PI, pio2),
            (t_c, w_c, TWO_PI, 0.0),
            (t_d, w_d, -TWO_PI, 0.0),
        ):
            nc.scalar.activation(
                out=t_t[:], in_=w_t[:],
                func=mybir.ActivationFunctionType.Sin,
                bias=bias if isinstance(bias, float) else bias[:],
                scale=scale,
            )

        # ---- per-b pipeline: table matmuls -> copy/DMA + exp/sum -> gather
        tb_ps = []
        tb_sb = []
        e_sb = []
        g_sb = []
        tbl_dram = []
        sums2 = sb.tile([128, 2], F32, tag="sums2")
        for b in range(BPC):
            bs = slice(256 * b, 256 * b + 256)
            t_ps = ps_t.tile([128, 512], F32, tag="tb")
            tb_ps.append(t_ps)
            # table[hi, lo] = sum_h cos(zhi)cos(zlo) - sin(zhi)sin(zlo)
            #   = A.T@B pairing: lhsT 128-col chunk of hi, rhs full 256 lo
            for c in range(2):
                cs = slice(256 * c, 256 * c + 256)
                hi_s = slice(256 * b + 128 * c, 256 * b + 128 * c + 128)
                nc.tensor.matmul(
                    out=t_ps[:, cs],
                    lhsT=t_b[:, hi_s],
                    rhs=t_a[:, bs],
                    start=True, stop=False,
                )
                nc.tensor.matmul(
                    out=t_ps[:, cs],
                    lhsT=t_d[:, hi_s],
                    rhs=t_c[:, bs],
                    start=False, stop=True,
                )
            # raw table to SBUF (DMA cannot read PSUM), then to DRAM
            t_sb = sb.tile([128, 512], F32, tag=f"tsb{b}")
            tb_sb.append(t_sb)
            nc.vector.tensor_copy(out=t_sb[:], in_=t_ps[:])
            tbl = dram.tile([65536, 1], F32, tag=f"tbl{b}")
            tbl_dram.append(tbl)
            for c in range(2):
                dst = tbl[32768 * c : 32768 * (c + 1), 0:1].rearrange(
                    "(p n) one -> p (n one)", p=128
                )
                nc.sync.dma_start(out=dst, in_=t_sb[:, 256 * c : 256 * c + 256])
            # gather: a [1, N, 1] dest makes the DGE emit one descriptor per
            # element, walking the offset AP partition-major (i = 128s + p).
            # One dest partition = one SBUF port (~6.4ns/element serialized),
            # so split into 4 calls on partitions {0,4,8,12} (+16 for b1),
            # which map to 8 distinct SBUF ports across the two batch rows.
            g_t = sb.tile([32, 1024], F32, tag=f"g{b}")
            for c in range(4):
                row = 4 * c + 16 * b
                nc.gpsimd.indirect_dma_start(
                    out=g_t[row : row + 1, :].rearrange(
                        "one (i x) -> one i x", x=1
                    ),
                    out_offset=None,
                    in_=tbl[:],
                    in_offset=bass.IndirectOffsetOnAxis(
                        ap=xv_sb[:, 32 * b + 8 * c : 32 * b + 8 * c + 8], axis=0
                    ),
                )
            # redistribute to g2[p, j] = value for t = 32p + j: with p =
            # 32c + q, t = 1024c + 32q + j lives at g_t[4c + 16b, 32q + j]
            g2_t = sb.tile([128, 32], F32, tag=f"g2{b}")
            g_sb.append(g2_t)
            for c in range(4):
                nc.sync.dma_start(
                    out=g2_t[32 * c : 32 * c + 32, :],
                    in_=g_t[4 * c + 16 * b : 4 * c + 16 * b + 1, :].rearrange(
                        "one (q j) -> one q j", j=32
                    ),
                )
            # exp + row sums for the partition function; EXP_SHIFT keeps
            # exp and the ln input inside fp32 / ACT-spline range
            e_t = sb.tile([128, 512], F32, tag=f"e{b}")
            e_sb.append(e_t)
            nc.scalar.activation(
                out=e_t[:], in_=t_ps[:], func=mybir.ActivationFunctionType.Exp,
                bias=neg_shift[:],
            )
            nc.vector.reduce_sum(
                sums2[:, b : b + 1], e_t[:], axis=mybir.AxisListType.X
            )

        # ---- logZ_b = ln(sum_v exp): partition sum via ones-matmul
        s_ps = ps_s.tile([2, 1], F32, tag="sps")
        nc.tensor.matmul(out=s_ps[:], lhsT=sums2[:], rhs=ones[:], start=True, stop=True)
        logz2 = sb.tile([2, 1], F32, tag="logz2")
        nc.scalar.activation(
            out=logz2[:], in_=s_ps[:], func=mybir.ActivationFunctionType.Ln,
        )

        # ---- out[b, t] = gathered - logZ_b, broadcast via negsel matmul
        for b in range(BPC):
            nz_ps = ps_s.tile([128, 1], F32, tag=f"nz{b}")
            nc.tensor.matmul(
                out=nz_ps[:],
                lhsT=negsel[:, 128 * b : 128 * b + 128],
                rhs=logz2[:],
                start=True, stop=True,
            )
            nz_sb = sb.tile([128, 1], F32, tag=f"nz{b}")
            nc.vector.tensor_copy(out=nz_sb[:], in_=nz_ps[:])
            o_t = sb.tile([128, 32], F32, tag=f"o{b}")
            nc.vector.tensor_scalar(
                out=o_t[:], in0=g_sb[b][:], scalar1=nz_sb[:], scalar2=-EXP_SHIFT,
                op0=mybir.AluOpType.add, op1=mybir.AluOpType.add,
            )
            dst = out[b, :].rearrange("(p j) -> p j", p=128)
            nc.sync.dma_start(out=dst, in_=o_t[:])

    nc.finalize()
    return nc


_NC = None


def _get_nc():
    global _NC
    if _NC is None:
        _NC = _build()
    return _NC


def _bf16_split(a):
    """Return (hi, lo) bf16 arrays with hi + lo ~= a (fp32)."""
    hi = a.astype(ml_dtypes.bfloat16)
    lo = (a - hi.astype(np.float32)).astype(ml_dtypes.bfloat16)
    return hi, lo


def _make_in_maps(x, W, r):
    x = np.asarray(x, dtype=np.int32)
    W = np.asarray(W, dtype=np.float32)
    r = np.asarray(r, dtype=np.float32)

    v = np.arange(256, dtype=np.int32)
    k = np.arange(8, dtype=np.int32)
    bitplanes = ((v[None, :] >> k[:, None]) & 1).astype(np.float32)  # [8, 256]
    bits = np.ones((18, 256), dtype=np.float32)
    bits[0:8] = bitplanes
    bits[8:16] = bitplanes
    bits = bits.astype(ml_dtypes.bfloat16)

    negsel = np.zeros((2, 256), dtype=np.float32)
    negsel[0, 0:128] = -1.0
    negsel[1, 128:256] = -1.0

    in_maps = []
    for core in range(N_CORES):
        wp = np.zeros((18, 512), dtype=ml_dtypes.bfloat16)
        xvs = []
        for b_loc in range(BPC):
            b = BPC * core + b_loc
            for half in range(2):
                g = 2 * b_loc + half
                cs = slice(128 * g, 128 * g + 128)
                w_t = W[b, :, 8 * half : 8 * half + 8].T * INV_2PI  # [8, 128]
                w_hi, w_lo = _bf16_split(w_t.astype(np.float32))
                wp[0:8, cs] = w_hi
                wp[8:16, cs] = w_lo
                if half == 1:
                    r_hi, r_lo = _bf16_split((r[b] * INV_2PI).astype(np.float32))
                    wp[16, cs] = r_hi
                    wp[17, cs] = r_lo
            xvs.append(x[b].reshape(32, 128).T)
        in_maps.append(
            {
                "wp": wp,
                "bits": bits,
                "xv": np.concatenate(xvs, axis=1).astype(np.int32),
                "negsel": negsel,
            }
        )
    return in_maps


def _run(x, W, r, trace=False):
    nc = _get_nc()
    in_maps = _make_in_maps(x, W, r)
    res = run_bass_kernel_spmd(nc, in_maps, core_ids=list(range(N_CORES)), trace=trace)
    out = np.concatenate([res.results[c]["out"] for c in range(N_CORES)], axis=0)
    return out.astype(np.float32), res


def kernel(x, W, r):
    out, _ = _run(x, W, r)
    return out


def kernel_traced(x, W, r):
    out, res = _run(x, W, r, trace=True)
    return out, res


# revision 28
# speedup vs baseline: 1.8770x; 1.0115x over previous
"""BIDE forward kernel for Trainium2, 8-core data parallel over B.

Math: logit[b, v] = sum_h cos(zlo[b, lo(v), h] + zhi[b, hi(v), h]) where
  zlo = bits(lo) @ W[:, :8].T          (lo = v & 255)
  zhi = bits(hi) @ W[:, 8:].T + r      (hi = v >> 8)
Using cos(p+q) = cos p cos q - sin p sin q, the [256, 256] logits table is
two K=128 matmuls over trig tables of shape [128 h, 256]:
  table = CloT.T @ ChiT - SloT.T @ ShiT   (per batch row)
logZ = log(sum_v exp(table)) (no max subtraction needed: |logit| <= 128 and
realized max ~40, so exp stays in fp32 range), and the output gather
out[b, t] = table[x[b, t]] - logZ is an indirect DMA from a DRAM copy of
the table.

Sin on the scalar engine only accepts [-pi, pi] (verified: it extrapolates
garbage outside), and the DVE has no mod op, so range reduction uses the
round-to-nearest f32->i32 conversion: the z matmul weights are pre-scaled
by 1/2pi so PSUM holds q = z/2pi; then qi = round(q + c'), w = q - qi, and
sin(z + 2pi c') = Sin(w; scale=2pi, bias=2pi c') with |2pi w + bias| <= pi.

Each core handles 2 of the 16 batch rows; zero cross-core communication.
"""

import numpy as np
import ml_dtypes
from contextlib import ExitStack

import concourse.bacc as bacc
import concourse.bass as bass
from concourse import mybir
from concourse.bass_utils import run_bass_kernel_spmd
from concourse.tile import TileContext

F32 = mybir.dt.float32
BF16 = mybir.dt.bfloat16
I32 = mybir.dt.int32

PI = float(np.float32(np.pi))
HALF_PI = float(np.float32(np.pi / 2.0))
TWO_PI = float(np.float32(2.0 * np.pi))
INV_2PI = 1.0 / (2.0 * np.pi)
# logits for these inputs peak at ~89 (exp overflows fp32) and the ACT Ln
# spline is only valid to 2^64; shift exp by a constant and add it back
EXP_SHIFT = 60.0

N_CORES = 8
B, H, T = 16, 128, 4096
BPC = B // N_CORES  # batch rows per core (2)


def _build():
    nc = bacc.Bacc("TRN2", target_bir_lowering=False, debug=False)

    # lhsT for the z matmuls, one 128-col group per (b, half):
    # rows 0-7 W_hi bits, 8-15 W_lo residual, 16 r_hi, 17 r_lo (hi half only)
    wp = nc.dram_tensor("wp", [18, 512], BF16, kind="ExternalInput")
    # bit-plane enumeration of v in [0, 256): rows 0-7 and 8-15 = (v>>k)&1,
    # rows 16-17 = 1.0 (carries r into zhi)
    bits = nc.dram_tensor("bits", [18, 256], BF16, kind="ExternalInput")
    # x indices for the gather: block b at cols [32b, 32b+32), laid out so
    # the indirect DMA's partition-major offset walk (i = s*128 + p) visits
    # t in order: xv[p, 32b + s] = x[b, 128s + p]
    xv = nc.dram_tensor("xv", [128, 64], I32, kind="ExternalInput")
    # negsel[k, 128b + m] = -1 if k == b else 0  (broadcast of -logZ_b)
    negsel_in = nc.dram_tensor("negsel", [2, 256], F32, kind="ExternalInput")
    out = nc.dram_tensor("out", [BPC, T], F32, kind="ExternalOutput")

    with ExitStack() as ctx:
        tc = ctx.enter_context(TileContext(nc))
        sb = ctx.enter_context(tc.tile_pool(name="sb", bufs=1))
        ps_z = ctx.enter_context(tc.tile_pool(name="ps_z", bufs=2, space="PSUM"))
        ps_t = ctx.enter_context(tc.tile_pool(name="ps_t", bufs=2, space="PSUM"))
        ps_s = ctx.enter_context(tc.tile_pool(name="ps_s", bufs=1, space="PSUM"))
        dram = ctx.enter_context(tc.tile_pool(name="dram", bufs=1, space="DRAM"))

        # ---- input loads
        wp_sb = sb.tile([18, 512], BF16, tag="wp")
        bits_sb = sb.tile([18, 256], BF16, tag="bits")
        xv_sb = sb.tile([128, 64], I32, tag="xv")
        nc.sync.dma_start(out=wp_sb[:], in_=wp[:])
        nc.sync.dma_start(out=bits_sb[:], in_=bits[:])
        nc.sync.dma_start(out=xv_sb[:], in_=xv[:])

        # ---- constants
        ones = sb.tile([128, 1], F32, tag="ones")
        nc.vector.memset(ones[:], 1.0)
        # per-partition bias tile for Sin (const-AP registry only has 0/1)
        pio2 = sb.tile([128, 1], F32, tag="pio2")
        nc.vector.memset(pio2[:], HALF_PI)
        neg_shift = sb.tile([128, 1], F32, tag="neg_shift")
        nc.vector.memset(neg_shift[:], -EXP_SHIFT)
        negsel = sb.tile([2, 256], F32, tag="negsel")
        nc.sync.dma_start(out=negsel[:], in_=negsel_in[:])

        # ---- q matmuls: q = z/2pi (weights pre-scaled by 1/2pi), [b0|b1]
        qlo_ps = ps_z.tile([128, 512], F32, tag="z")
        qhi_ps = ps_z.tile([128, 512], F32, tag="z")
        for b in range(BPC):
            nc.tensor.matmul(
                out=qlo_ps[:, 256 * b : 256 * b + 256],
                lhsT=wp_sb[:, 128 * (2 * b) : 128 * (2 * b) + 128],
                rhs=bits_sb[:],
                start=True,
                stop=True,
            )
            nc.tensor.matmul(
                out=qhi_ps[:, 256 * b : 256 * b + 256],
                lhsT=wp_sb[:, 128 * (2 * b + 1) : 128 * (2 * b + 1) + 128],
                rhs=bits_sb[:],
                start=True,
                stop=True,
            )

        # ---- range reduction + trig, per batch row so b0's table chain
        # starts as early as possible: qi = round(q + c') (f32->i32 rounds
        # to nearest), w = q - qi, then Sin(scale*w + bias), |arg| <= pi
        F32R = mybir.dt.float32r
        t_a = sb.tile([128, 512], F32R, tag="t_a")  # cos(zlo)
        t_b = sb.tile([128, 512], F32R, tag="t_b")  # cos(zhi)
        t_c = sb.tile([128, 512], F32R, tag="t_c")  # sin(zlo)
        t_d = sb.tile([128, 512], F32R, tag="t_d")  # -sin(zhi) (scale=-2pi)

        def trig_for_b(b):
            bs = slice(256 * b, 256 * b + 256)
            for i, (t_t, q_ps, cp, scale, bias) in enumerate((
                (t_a, qlo_ps, 0.25, TWO_PI, None),
                (t_b, qhi_ps, 0.25, TWO_PI, None),
                (t_c, qlo_ps, 0.0, TWO_PI, 0.0),
                (t_d, qhi_ps, 0.0, -TWO_PI, 0.0),
            )):
                qi_t = sb.tile([128, 256], I32, tag=f"qi{i}{b}")
                if cp == 0.0:
                    nc.vector.tensor_copy(out=qi_t[:], in_=q_ps[:, bs])
                else:
                    nc.vector.tensor_scalar(
                        out=qi_t[:], in0=q_ps[:, bs], scalar1=cp, scalar2=None,
                        op0=mybir.AluOpType.add,
                    )
                w_t = sb.tile([128, 256], F32, tag=f"w{i}{b}")
                nc.vector.tensor_tensor(
                    out=w_t[:], in0=q_ps[:, bs], in1=qi_t[:],
                    op=mybir.AluOpType.subtract,
                )
                nc.scalar.activation(
                    out=t_t[:, bs], in_=w_t[:],
                    func=mybir.ActivationFunctionType.Sin,
                    bias=bias if isinstance(bias, float) else pio2[:],
                    scale=scale,
                )

        # ---- per-b pipeline: table matmuls -> copy/DMA + exp/sum -> gather
        tb_ps = []
        tb_sb = []
        e_sb = []
        g_sb = []
        tbl_dram = []
        sums2 = sb.tile([128, 2], F32, tag="sums2")
        for b in range(BPC):
            trig_for_b(b)
            bs = slice(256 * b, 256 * b + 256)
            t_ps = ps_t.tile([128, 512], F32, tag="tb")
            tb_ps.append(t_ps)
            # table[hi, lo] = sum_h cos(zhi)cos(zlo) - sin(zhi)sin(zlo)
            #   = A.T@B pairing: lhsT 128-col chunk of hi, rhs full 256 lo
            for c in range(2):
                cs = slice(256 * c, 256 * c + 256)
                hi_s = slice(256 * b + 128 * c, 256 * b + 128 * c + 128)
                nc.tensor.matmul(
                    out=t_ps[:, cs],
                    lhsT=t_b[:, hi_s],
                    rhs=t_a[:, bs],
                    start=True, stop=False,
                )
                nc.tensor.matmul(
                    out=t_ps[:, cs],
                    lhsT=t_d[:, hi_s],
                    rhs=t_c[:, bs],
                    start=False, stop=True,
                )
            # raw table to SBUF (DMA cannot read PSUM), then to DRAM
            t_sb = sb.tile([128, 512], F32, tag=f"tsb{b}")
            tb_sb.append(t_sb)
            nc.vector.tensor_copy(out=t_sb[:], in_=t_ps[:])
            tbl = dram.tile([65536, 1], F32, tag=f"tbl{b}")
            tbl_dram.append(tbl)
            for c in range(2):
                dst = tbl[32768 * c : 32768 * (c + 1), 0:1].rearrange(
                    "(p n) one -> p (n one)", p=128
                )
                nc.sync.dma_start(out=dst, in_=t_sb[:, 256 * c : 256 * c + 256])
            # gather: a [1, N, 1] dest makes the DGE emit one descriptor per
            # element, walking the offset AP partition-major (i = 128s + p).
            # One dest partition = one SBUF port (~6.4ns/element serialized),
            # so split into 4 calls on partitions {0,4,8,12} (+16 for b1),
            # which map to 8 distinct SBUF ports across the two batch rows.
            g_t = sb.tile([32, 1024], F32, tag=f"g{b}")
            for c in range(4):
                row = 4 * c + 16 * b
                nc.gpsimd.indirect_dma_start(
                    out=g_t[row : row + 1, :].rearrange(
                        "one (i x) -> one i x", x=1
                    ),
                    out_offset=None,
                    in_=tbl[:],
                    in_offset=bass.IndirectOffsetOnAxis(
                        ap=xv_sb[:, 32 * b + 8 * c : 32 * b + 8 * c + 8], axis=0
                    ),
                )
            # redistribute to g2[p, j] = value for t = 32p + j: with p =
            # 32c + q, t = 1024c + 32q + j lives at g_t[4c + 16b, 32q + j]
            g2_t = sb.tile([128, 32], F32, tag=f"g2{b}")
            g_sb.append(g2_t)
            for c in range(4):
                nc.sync.dma_start(
                    out=g2_t[32 * c : 32 * c + 32, :],
                    in_=g_t[4 * c + 16 * b : 4 * c + 16 * b + 1, :].rearrange(
                        "one (q j) -> one q j", j=32
                    ),
                )
            # exp + row sums for the partition function; EXP_SHIFT keeps
            # exp and the ln input inside fp32 / ACT-spline range
            e_t = sb.tile([128, 512], F32, tag=f"e{b}")
            e_sb.append(e_t)
            nc.scalar.activation(
                out=e_t[:], in_=t_ps[:], func=mybir.ActivationFunctionType.Exp,
                bias=neg_shift[:],
            )
            nc.vector.reduce_sum(
                sums2[:, b : b + 1], e_t[:], axis=mybir.AxisListType.X
            )

        # ---- logZ_b = ln(sum_v exp): partition sum via ones-matmul
        s_ps = ps_s.tile([2, 1], F32, tag="sps")
        nc.tensor.matmul(out=s_ps[:], lhsT=sums2[:], rhs=ones[:], start=True, stop=True)
        logz2 = sb.tile([2, 1], F32, tag="logz2")
        nc.scalar.activation(
            out=logz2[:], in_=s_ps[:], func=mybir.ActivationFunctionType.Ln,
        )

        # ---- out[b, t] = gathered - logZ_b, broadcast via negsel matmul
        for b in range(BPC):
            nz_ps = ps_s.tile([128, 1], F32, tag=f"nz{b}")
            nc.tensor.matmul(
                out=nz_ps[:],
                lhsT=negsel[:, 128 * b : 128 * b + 128],
                rhs=logz2[:],
                start=True, stop=True,
            )
            nz_sb = sb.tile([128, 1], F32, tag=f"nz{b}")
            nc.vector.tensor_copy(out=nz_sb[:], in_=nz_ps[:])
            o_t = sb.tile([128, 32], F32, tag=f"o{b}")
            nc.vector.tensor_scalar(
                out=o_t[:], in0=g_sb[b][:], scalar1=nz_sb[:], scalar2=-EXP_SHIFT,
                op0=mybir.AluOpType.add, op1=mybir.AluOpType.add,
            )
            dst = out[b, :].rearrange("(p j) -> p j", p=128)
            nc.sync.dma_start(out=dst, in_=o_t[:])

    nc.finalize()
    return nc


_NC = None


def _get_nc():
    global _NC
    if _NC is None:
        _NC = _build()
    return _NC


def _bf16_split(a):
    """Return (hi, lo) bf16 arrays with hi + lo ~= a (fp32)."""
    hi = a.astype(ml_dtypes.bfloat16)
    lo = (a - hi.astype(np.float32)).astype(ml_dtypes.bfloat16)
    return hi, lo


def _make_in_maps(x, W, r):
    x = np.asarray(x, dtype=np.int32)
    W = np.asarray(W, dtype=np.float32)
    r = np.asarray(r, dtype=np.float32)

    v = np.arange(256, dtype=np.int32)
    k = np.arange(8, dtype=np.int32)
    bitplanes = ((v[None, :] >> k[:, None]) & 1).astype(np.float32)  # [8, 256]
    bits = np.ones((18, 256), dtype=np.float32)
    bits[0:8] = bitplanes
    bits[8:16] = bitplanes
    bits = bits.astype(ml_dtypes.bfloat16)

    negsel = np.zeros((2, 256), dtype=np.float32)
    negsel[0, 0:128] = -1.0
    negsel[1, 128:256] = -1.0

    in_maps = []
    for core in range(N_CORES):
        wp = np.zeros((18, 512), dtype=ml_dtypes.bfloat16)
        xvs = []
        for b_loc in range(BPC):
            b = BPC * core + b_loc
            for half in range(2):
                g = 2 * b_loc + half
                cs = slice(128 * g, 128 * g + 128)
                w_t = W[b, :, 8 * half : 8 * half + 8].T * INV_2PI  # [8, 128]
                w_hi, w_lo = _bf16_split(w_t.astype(np.float32))
                wp[0:8, cs] = w_hi
                wp[8:16, cs] = w_lo
                if half == 1:
                    r_hi, r_lo = _bf16_split((r[b] * INV_2PI).astype(np.float32))
                    wp[16, cs] = r_hi
                    wp[17, cs] = r_lo
            xvs.append(x[b].reshape(32, 128).T)
        in_maps.append(
            {
                "wp": wp,
                "bits": bits,
                "xv": np.concatenate(xvs, axis=1).astype(np.int32),
                "negsel": negsel,
            }
        )
    return in_maps


def _run(x, W, r, trace=False):
    nc = _get_nc()
    in_maps = _make_in_maps(x, W, r)
    res = run_bass_kernel_spmd(nc, in_maps, core_ids=list(range(N_CORES)), trace=trace)
    out = np.concatenate([res.results[c]["out"] for c in range(N_CORES)], axis=0)
    return out.astype(np.float32), res


def kernel(x, W, r):
    out, _ = _run(x, W, r)
    return out


def kernel_traced(x, W, r):
    out, res = _run(x, W, r, trace=True)
    return out, res


# revision 29
# speedup vs baseline: 1.9074x; 1.0162x over previous
"""BIDE forward kernel for Trainium2, 8-core data parallel over B.

Math: logit[b, v] = sum_h cos(zlo[b, lo(v), h] + zhi[b, hi(v), h]) where
  zlo = bits(lo) @ W[:, :8].T          (lo = v & 255)
  zhi = bits(hi) @ W[:, 8:].T + r      (hi = v >> 8)
Using cos(p+q) = cos p cos q - sin p sin q, the [256, 256] logits table is
two K=128 matmuls over trig tables of shape [128 h, 256]:
  table = CloT.T @ ChiT - SloT.T @ ShiT   (per batch row)
logZ = EXP_SHIFT + log(sum_v exp(table - EXP_SHIFT)) (constant shift: the
realized max logit is ~89, exp would overflow fp32 and the ACT Ln spline
is only valid to 2^64), and the output gather out[b, t] = table[x[b, t]]
- logZ is a per-element indirect DMA from a DRAM copy of the table.

Sin on the scalar engine only accepts [-pi, pi] (verified: it extrapolates
garbage outside), and the DVE has no mod op, so range reduction uses the
round-to-nearest f32->i32 conversion: the z matmul weights are pre-scaled
by 1/2pi so PSUM holds q = z/2pi; then qi = round(q + c'), w = q - qi, and
sin(z + 2pi c') = Sin(w; scale=2pi, bias=2pi c') with |2pi w + bias| <= pi.

Each core handles 2 of the 16 batch rows; zero cross-core communication.
"""

import numpy as np
import ml_dtypes
from contextlib import ExitStack

import concourse.bacc as bacc
import concourse.bass as bass
from concourse import mybir
from concourse.bass_utils import run_bass_kernel_spmd
from concourse.tile import TileContext

F32 = mybir.dt.float32
BF16 = mybir.dt.bfloat16
I32 = mybir.dt.int32

PI = float(np.float32(np.pi))
HALF_PI = float(np.float32(np.pi / 2.0))
TWO_PI = float(np.float32(2.0 * np.pi))
INV_2PI = 1.0 / (2.0 * np.pi)
# logits for these inputs peak at ~89 (exp overflows fp32) and the ACT Ln
# spline is only valid to 2^64; shift exp by a constant and add it back
EXP_SHIFT = 60.0

N_CORES = 8
B, H, T = 16, 128, 4096
BPC = B // N_CORES  # batch rows per core (2)


def _build():
    nc = bacc.Bacc("TRN2", target_bir_lowering=False, debug=False)

    # lhsT for the z matmuls, one 128-col group per (b, half):
    # rows 0-7 W_hi bits, 8-15 W_lo residual, 16 r_hi, 17 r_lo (hi half only)
    wp = nc.dram_tensor("wp", [18, 512], BF16, kind="ExternalInput")
    # bit-plane enumeration of v in [0, 256): rows 0-7 and 8-15 = (v>>k)&1,
    # rows 16-17 = 1.0 (carries r into zhi)
    bits = nc.dram_tensor("bits", [18, 256], BF16, kind="ExternalInput")
    # x indices for the gather: block b at cols [32b, 32b+32), laid out so
    # the indirect DMA's partition-major offset walk (i = s*128 + p) visits
    # t in order: xv[p, 32b + s] = x[b, 128s + p]
    xv = nc.dram_tensor("xv", [128, 64], I32, kind="ExternalInput")
    # negsel[k, 128b + m] = -1 if k == b else 0  (broadcast of -logZ_b)
    negsel_in = nc.dram_tensor("negsel", [2, 256], F32, kind="ExternalInput")
    out = nc.dram_tensor("out", [BPC, T], F32, kind="ExternalOutput")

    with ExitStack() as ctx:
        tc = ctx.enter_context(TileContext(nc))
        sb = ctx.enter_context(tc.tile_pool(name="sb", bufs=1))
        ps_z = ctx.enter_context(tc.tile_pool(name="ps_z", bufs=2, space="PSUM"))
        ps_t = ctx.enter_context(tc.tile_pool(name="ps_t", bufs=2, space="PSUM"))
        ps_s = ctx.enter_context(tc.tile_pool(name="ps_s", bufs=1, space="PSUM"))
        dram = ctx.enter_context(tc.tile_pool(name="dram", bufs=1, space="DRAM"))

        # ---- input loads
        wp_sb = sb.tile([18, 512], BF16, tag="wp")
        bits_sb = sb.tile([18, 256], BF16, tag="bits")
        xv_sb = sb.tile([128, 64], I32, tag="xv")
        nc.sync.dma_start(out=wp_sb[:], in_=wp[:])
        nc.sync.dma_start(out=bits_sb[:], in_=bits[:])
        nc.sync.dma_start(out=xv_sb[:], in_=xv[:])

        # ---- constants
        ones = sb.tile([128, 1], F32, tag="ones")
        nc.vector.memset(ones[:], 1.0)
        # per-partition bias tile for Sin (const-AP registry only has 0/1)
        pio2 = sb.tile([128, 1], F32, tag="pio2")
        nc.vector.memset(pio2[:], HALF_PI)
        neg_shift = sb.tile([128, 1], F32, tag="neg_shift")
        nc.vector.memset(neg_shift[:], -EXP_SHIFT)
        negsel = sb.tile([2, 256], F32, tag="negsel")
        nc.sync.dma_start(out=negsel[:], in_=negsel_in[:])

        # ---- q matmuls: q = z/2pi (weights pre-scaled by 1/2pi), [b0|b1]
        qlo_ps = ps_z.tile([128, 512], F32, tag="z")
        qhi_ps = ps_z.tile([128, 512], F32, tag="z")
        for b in range(BPC):
            nc.tensor.matmul(
                out=qlo_ps[:, 256 * b : 256 * b + 256],
                lhsT=wp_sb[:, 128 * (2 * b) : 128 * (2 * b) + 128],
                rhs=bits_sb[:],
                start=True,
                stop=True,
            )
            nc.tensor.matmul(
                out=qhi_ps[:, 256 * b : 256 * b + 256],
                lhsT=wp_sb[:, 128 * (2 * b + 1) : 128 * (2 * b + 1) + 128],
                rhs=bits_sb[:],
                start=True,
                stop=True,
            )

        # ---- range reduction + trig, per batch row so b0's table chain
        # starts as early as possible: qi = round(q + c') (f32->i32 rounds
        # to nearest), w = q - qi, then Sin(scale*w + bias), |arg| <= pi
        F32R = mybir.dt.float32r
        t_a = sb.tile([128, 512], F32R, tag="t_a")  # cos(zlo)
        t_b = sb.tile([128, 512], F32R, tag="t_b")  # cos(zhi)
        t_c = sb.tile([128, 512], F32R, tag="t_c")  # sin(zlo)
        t_d = sb.tile([128, 512], F32R, tag="t_d")  # -sin(zhi) (scale=-2pi)

        def trig_for_b(b):
            bs = slice(256 * b, 256 * b + 256)
            for i, (t_t, q_ps, cp, scale, bias) in enumerate((
                (t_a, qlo_ps, 0.25, TWO_PI, None),
                (t_b, qhi_ps, 0.25, TWO_PI, None),
                (t_c, qlo_ps, 0.0, TWO_PI, 0.0),
                (t_d, qhi_ps, 0.0, -TWO_PI, 0.0),
            )):
                qi_t = sb.tile([128, 256], I32, tag=f"qi{i}{b}")
                if cp == 0.0:
                    nc.vector.tensor_copy(out=qi_t[:], in_=q_ps[:, bs])
                else:
                    nc.vector.tensor_scalar(
                        out=qi_t[:], in0=q_ps[:, bs], scalar1=cp, scalar2=None,
                        op0=mybir.AluOpType.add,
                    )
                w_t = sb.tile([128, 256], F32, tag=f"w{i}{b}")
                nc.vector.tensor_tensor(
                    out=w_t[:], in0=q_ps[:, bs], in1=qi_t[:],
                    op=mybir.AluOpType.subtract,
                )
                nc.scalar.activation(
                    out=t_t[:, bs], in_=w_t[:],
                    func=mybir.ActivationFunctionType.Sin,
                    bias=bias if isinstance(bias, float) else pio2[:],
                    scale=scale,
                )

        # ---- per-b pipeline: table matmuls -> copy/DMA + exp/sum -> gather
        tb_ps = []
        tb_sb = []
        e_sb = []
        g_sb = []
        tbl_dram = []
        sums2 = sb.tile([128, 2], F32, tag="sums2")
        for b in range(BPC):
            trig_for_b(b)
            bs = slice(256 * b, 256 * b + 256)
            t_ps = ps_t.tile([128, 512], F32, tag="tb")
            tb_ps.append(t_ps)
            # table[hi, lo] = sum_h cos(zhi)cos(zlo) - sin(zhi)sin(zlo)
            #   = A.T@B pairing: lhsT 128-col chunk of hi, rhs full 256 lo
            for c in range(2):
                cs = slice(256 * c, 256 * c + 256)
                hi_s = slice(256 * b + 128 * c, 256 * b + 128 * c + 128)
                nc.tensor.matmul(
                    out=t_ps[:, cs],
                    lhsT=t_b[:, hi_s],
                    rhs=t_a[:, bs],
                    start=True, stop=False,
                )
                nc.tensor.matmul(
                    out=t_ps[:, cs],
                    lhsT=t_d[:, hi_s],
                    rhs=t_c[:, bs],
                    start=False, stop=True,
                )
            # raw table to SBUF (DMA cannot read PSUM), then to DRAM
            t_sb = sb.tile([128, 512], F32, tag=f"tsb{b}")
            tb_sb.append(t_sb)
            nc.vector.tensor_copy(out=t_sb[:], in_=t_ps[:])
            tbl = dram.tile([65536, 1], F32, tag=f"tbl{b}")
            tbl_dram.append(tbl)
            for c in range(2):
                dst = tbl[32768 * c : 32768 * (c + 1), 0:1].rearrange(
                    "(p n) one -> p (n one)", p=128
                )
                nc.sync.dma_start(out=dst, in_=t_sb[:, 256 * c : 256 * c + 256])
            # gather: a [1, N, 1] dest makes the DGE emit one descriptor per
            # element, walking the offset AP partition-major (i = 128s + p).
            # One dest partition = one SBUF port (~6.4ns/element serialized),
            # so split into 4 calls on partitions {0,4,8,12} (+16 for b1),
            # which map to 8 distinct SBUF ports across the two batch rows.
            g_t = sb.tile([32, 1024], F32, tag=f"g{b}")
            for c in range(4):
                row = 4 * c + 16 * b
                nc.gpsimd.indirect_dma_start(
                    out=g_t[row : row + 1, :].rearrange(
                        "one (i x) -> one i x", x=1
                    ),
                    out_offset=None,
                    in_=tbl[:],
                    in_offset=bass.IndirectOffsetOnAxis(
                        ap=xv_sb[:, 32 * b + 8 * c : 32 * b + 8 * c + 8], axis=0
                    ),
                )
            # redistribute to g2[p, j] = value for t = 32p + j: with p =
            # 32c + q, t = 1024c + 32q + j lives at g_t[4c + 16b, 32q + j]
            g2_t = sb.tile([128, 32], F32, tag=f"g2{b}")
            g_sb.append(g2_t)
            for c in range(4):
                nc.sync.dma_start(
                    out=g2_t[32 * c : 32 * c + 32, :],
                    in_=g_t[4 * c + 16 * b : 4 * c + 16 * b + 1, :].rearrange(
                        "one (q j) -> one q j", j=32
                    ),
                )
            # exp + row sums for the partition function; EXP_SHIFT keeps
            # exp and the ln input inside fp32 / ACT-spline range
            e_t = sb.tile([128, 512], F32, tag=f"e{b}")
            e_sb.append(e_t)
            nc.scalar.activation(
                out=e_t[:], in_=t_ps[:], func=mybir.ActivationFunctionType.Exp,
                bias=neg_shift[:],
            )
            nc.vector.reduce_sum(
                sums2[:, b : b + 1], e_t[:], axis=mybir.AxisListType.X
            )

        # ---- logZ_b = ln(sum_v exp): partition sum via ones-matmul
        s_ps = ps_s.tile([2, 1], F32, tag="sps")
        nc.tensor.matmul(out=s_ps[:], lhsT=sums2[:], rhs=ones[:], start=True, stop=True)
        logz2 = sb.tile([2, 1], F32, tag="logz2")
        nc.scalar.activation(
            out=logz2[:], in_=s_ps[:], func=mybir.ActivationFunctionType.Ln,
        )

        # ---- out[b, t] = gathered - logZ_b, broadcast via negsel matmul
        for b in range(BPC):
            nz_ps = ps_s.tile([128, 1], F32, tag=f"nz{b}")
            nc.tensor.matmul(
                out=nz_ps[:],
                lhsT=negsel[:, 128 * b : 128 * b + 128],
                rhs=logz2[:],
                start=True, stop=True,
            )
            nz_sb = sb.tile([128, 1], F32, tag=f"nz{b}")
            nc.vector.tensor_copy(out=nz_sb[:], in_=nz_ps[:])
            o_t = sb.tile([128, 32], F32, tag=f"o{b}")
            nc.vector.tensor_scalar(
                out=o_t[:], in0=g_sb[b][:], scalar1=nz_sb[:], scalar2=-EXP_SHIFT,
                op0=mybir.AluOpType.add, op1=mybir.AluOpType.add,
            )
            dst = out[b, :].rearrange("(p j) -> p j", p=128)
            nc.sync.dma_start(out=dst, in_=o_t[:])

    nc.finalize()
    return nc


_NC = None


def _get_nc():
    global _NC
    if _NC is None:
        _NC = _build()
    return _NC


def _bf16_split(a):
    """Return (hi, lo) bf16 arrays with hi + lo ~= a (fp32)."""
    hi = a.astype(ml_dtypes.bfloat16)
    lo = (a - hi.astype(np.float32)).astype(ml_dtypes.bfloat16)
    return hi, lo


def _make_in_maps(x, W, r):
    x = np.asarray(x, dtype=np.int32)
    W = np.asarray(W, dtype=np.float32)
    r = np.asarray(r, dtype=np.float32)

    v = np.arange(256, dtype=np.int32)
    k = np.arange(8, dtype=np.int32)
    bitplanes = ((v[None, :] >> k[:, None]) & 1).astype(np.float32)  # [8, 256]
    bits = np.ones((18, 256), dtype=np.float32)
    bits[0:8] = bitplanes
    bits[8:16] = bitplanes
    bits = bits.astype(ml_dtypes.bfloat16)

    negsel = np.zeros((2, 256), dtype=np.float32)
    negsel[0, 0:128] = -1.0
    negsel[1, 128:256] = -1.0

    in_maps = []
    for core in range(N_CORES):
        wp = np.zeros((18, 512), dtype=ml_dtypes.bfloat16)
        xvs = []
        for b_loc in range(BPC):
            b = BPC * core + b_loc
            for half in range(2):
                g = 2 * b_loc + half
                cs = slice(128 * g, 128 * g + 128)
                w_t = W[b, :, 8 * half : 8 * half + 8].T * INV_2PI  # [8, 128]
                w_hi, w_lo = _bf16_split(w_t.astype(np.float32))
                wp[0:8, cs] = w_hi
                wp[8:16, cs] = w_lo
                if half == 1:
                    r_hi, r_lo = _bf16_split((r[b] * INV_2PI).astype(np.float32))
                    wp[16, cs] = r_hi
                    wp[17, cs] = r_lo
            xvs.append(x[b].reshape(32, 128).T)
        in_maps.append(
            {
                "wp": wp,
                "bits": bits,
                "xv": np.concatenate(xvs, axis=1).astype(np.int32),
                "negsel": negsel,
            }
        )
    return in_maps


def _run(x, W, r, trace=False):
    nc = _get_nc()
    in_maps = _make_in_maps(x, W, r)
    res = run_bass_kernel_spmd(nc, in_maps, core_ids=list(range(N_CORES)), trace=trace)
    out = np.concatenate([res.results[c]["out"] for c in range(N_CORES)], axis=0)
    return out.astype(np.float32), res


def kernel(x, W, r):
    out, _ = _run(x, W, r)
    return out


def kernel_traced(x, W, r):
    out, res = _run(x, W, r, trace=True)
    return out, res
